# revision 4
# baseline (speedup 1.0000x reference)
"""Bass/Trainium2 kernel for naive causal multi-head attention.

Problem: B=4, S=2048, E=1024, H=16, DH=64 (fp32 in/out).

Sharding (8 NeuronCores): core c handles batch b = c//2 and head group
g = c%2 (heads 8g..8g+7).  Each core computes its 8 heads' attention for
its batch plus the partial out-projection through its 512 columns of the
concat dim; the host sums the two partial outputs per batch.

Device-side dataflow (all matmuls bf16, PSUM fp32):
  xT   = transpose(cast_bf16(x))                    (cast-DMA + xbar transpose)
  q/kT = Wqk_h @ xT   -> [128(q64|k64), S] per head (PE)
  v'   = x @ Wv^T (+ ones column per head)          (PE)
  sT   = kT_blk^T q  -> [128 keys, 512 q] blocks    (PE, causal-skipped)
  p    = exp(sT/8)   (PSUM->SBUF, diag-masked)      (ACT exp, DVE mask)
  oT   = v'^T p      -> [65, 512]  (row 64 = sums)  (PE, accumulated)
  oT  /= sums        (recip + ones-bcast matmul)    (DVE/PE/ACT)
  out  = concatT^T @ WoT                            (PE)
"""

import numpy as np

import concourse.bacc as bacc
import concourse.mybir as mybir
from concourse.tile import TileContext
from concourse.bass_utils import run_bass_kernel_spmd

F32 = mybir.dt.float32
BF16 = mybir.dt.bfloat16
EXP = mybir.ActivationFunctionType.Exp

N_CORES = 8


def build_nc(S=2048, E=1024, HPC=8, DH=64):
    """Build the per-core Bass program (identical on all cores)."""
    NQ = 512                      # query-tile width
    nst = S // 128                # s-tiles (key tiles)
    nec = E // 128                # e chunks (contraction tiles)
    nqt = S // NQ                 # query tiles
    HD = HPC * DH                 # local concat width (512)
    ncc = HD // 128               # concat chunks (4)
    nnc = E // 512                # out-proj N chunks (2)
    assert NQ == 512 and S % 512 == 0

    nc = bacc.Bacc("TRN2", target_bir_lowering=False, debug=False,
                   num_devices=N_CORES)

    xb = nc.dram_tensor("xb", [S, E], F32, kind="ExternalInput")
    wqkt = nc.dram_tensor("wqkt", [128, HPC, nec, 2 * DH], F32,
                          kind="ExternalInput")
    wvt = nc.dram_tensor("wvt", [128, nec, HD], F32, kind="ExternalInput")
    wot = nc.dram_tensor("wot", [128, ncc, E], F32, kind="ExternalInput")
    maskab = nc.dram_tensor("maskab", [128, 2 * 1024], F32,
                            kind="ExternalInput")
    out = nc.dram_tensor("out", [S, E], F32, kind="ExternalOutput")

    with TileContext(nc) as tc:
        with (
            tc.tile_pool(name="persist", bufs=1) as persist,
            tc.tile_pool(name="dramp", bufs=1, space="DRAM") as dramp,
            tc.tile_pool(name="qkp", bufs=2) as qkp,
            tc.tile_pool(name="ktp", bufs=2) as ktp,
            tc.tile_pool(name="ptp", bufs=3) as ptp,
            tc.tile_pool(name="othp", bufs=2) as othp,
            tc.tile_pool(name="outp", bufs=2) as outp,
            tc.tile_pool(name="miscp", bufs=4) as miscp,
            tc.tile_pool(name="ps_big", bufs=2, space="PSUM") as ps_big,
            tc.tile_pool(name="ps_o", bufs=2, space="PSUM") as ps_o,
            tc.tile_pool(name="ps_sm", bufs=2, space="PSUM") as ps_sm,
        ):
            # ---- persistent SBUF tensors ----
            xT = persist.tile([128, nec, S], BF16)
            wqk = persist.tile([128, HPC, nec, 2 * DH], BF16)
            wv = persist.tile([128, nec, HD], BF16)
            wo = persist.tile([128, ncc, E], BF16)
            vS = persist.tile([128, nst, HPC * (DH + 1)], BF16)
            cT = persist.tile([128, ncc, S], BF16)
            mk = persist.tile([128, 2 * 1024], BF16)
            ones1 = persist.tile([1, DH], BF16)

            x_bf = dramp.tile([S, E], BF16)

            # ---- phase A: loads, casts, x transpose ----
            nc.vector.memset(ones1, 1.0)
            nc.vector.memset(vS, 1.0)
            nc.gpsimd.dma_start(out=wqk, in_=wqkt[:, :, :, :])
            nc.gpsimd.dma_start(out=wv, in_=wvt[:, :, :])
            nc.gpsimd.dma_start(out=wo, in_=wot[:, :, :])
            nc.gpsimd.dma_start(out=mk, in_=maskab[:, :])
            for j in range(4):
                r = slice(j * (S // 4), (j + 1) * (S // 4))
                nc.gpsimd.dma_start(out=x_bf[r, :], in_=xb[r, :])
            for ec in range(nec):
                nc.sync.dma_start_transpose(
                    out=xT[:, ec], in_=x_bf[:, ec * 128:(ec + 1) * 128])

            # ---- phase B: v projection (all heads), with ones column ----
            for st in range(nst):
                pv = ps_sm.tile([128, HD], F32, tag="sm")
                for ec in range(nec):
                    nc.tensor.matmul(
                        pv, lhsT=xT[:, ec, st * 128:(st + 1) * 128],
                        rhs=wv[:, ec], start=(ec == 0), stop=(ec == nec - 1))
                nc.vector.tensor_copy(
                    out=vS[:, st].rearrange("p (h m) -> p h m",
                                            m=DH + 1)[:, :, 0:DH],
                    in_=pv.rearrange("p (h m) -> p h m", m=DH))

            # ---- phase C: per head ----
            for h in range(HPC):
                # C1: q/k projection -> qk [128 (q64|k64), S]
                qk = qkp.tile([128, S], BF16, tag="qk")
                for sc in range(nqt):
                    pqk = ps_sm.tile([128, 512], F32, tag="sm")
                    for ec in range(nec):
                        nc.tensor.matmul(
                            pqk, lhsT=wqk[:, h, ec],
                            rhs=xT[:, ec, sc * 512:(sc + 1) * 512],
                            start=(ec == 0), stop=(ec == nec - 1))
                    nc.vector.tensor_copy(
                        out=qk[:, sc * 512:(sc + 1) * 512], in_=pqk)
                # realign k rows 64:128 -> partitions 0:64 (SBUF->SBUF DMA)
                kt_sb = ktp.tile([64, S], BF16, tag="kt")
                nc.sync.dma_start(out=kt_sb, in_=qk[64:128, :])

                oth = othp.tile([64, S], BF16, tag="oth")
                # C2: attention per query tile
                for qt in range(nqt):
                    po = ps_o.tile([DH + 1, 512], F32, tag="o")
                    ngrp = 2 * qt + 2
                    for g in range(ngrp):
                        ps2 = ps_big.tile([128, 1024], F32, tag="sbig")
                        for kk in range(2):
                            kt = 2 * g + kk
                            nc.tensor.matmul(
                                ps2[:, kk * 512:(kk + 1) * 512],
                                lhsT=kt_sb[:, kt * 128:(kt + 1) * 128],
                                rhs=qk[0:64, qt * 512:(qt + 1) * 512],
                                start=True, stop=True)
                        pt = ptp.tile([128, 1024], BF16, tag="pt")
                        nc.scalar.activation(out=pt, in_=ps2, func=EXP,
                                             scale=0.125)
                        if g >= 2 * qt:  # diagonal groups need causal mask
                            mi = g - 2 * qt
                            nc.vector.tensor_mul(
                                pt, pt, mk[:, mi * 1024:(mi + 1) * 1024])
                        for kk in range(2):
                            kt = 2 * g + kk
                            nc.tensor.matmul(
                                po,
                                lhsT=vS[:, kt, h * (DH + 1):(h + 1) * (DH + 1)],
                                rhs=pt[:, kk * 512:(kk + 1) * 512],
                                start=(g == 0 and kk == 0),
                                stop=(g == ngrp - 1 and kk == 1))
                    # normalize: oT[d, q] * (1 / sums[q])
                    rec = miscp.tile([1, 512], BF16, tag="rec")
                    with nc.allow_low_precision("softmax denom recip in bf16"):
                        nc.vector.reciprocal(rec, po[DH:DH + 1, :])
                    pb = ps_sm.tile([64, 512], F32, tag="sm")
                    nc.tensor.matmul(pb, lhsT=ones1, rhs=rec,
                                     start=True, stop=True)
                    bc = miscp.tile([64, 512], BF16, tag="bc")
                    nc.scalar.copy(bc, pb)
                    nc.vector.tensor_mul(
                        oth[:, qt * 512:(qt + 1) * 512], po[0:DH, :], bc)
                # place head output rows into concatT (partition shift DMA)
                nc.sync.dma_start(
                    out=cT[64 * (h % 2):64 * (h % 2) + 64, h // 2, :],
                    in_=oth)

            # ---- phase D: partial out-projection ----
            nw = min(512, E)
            for st in range(nst):
                pd = ps_big.tile([128, min(E, 1024)], F32, tag="sbig")
                for n2 in range(E // nw):
                    for c in range(ncc):
                        nc.tensor.matmul(
                            pd[:, n2 * nw:(n2 + 1) * nw],
                            lhsT=cT[:, c, st * 128:(st + 1) * 128],
                            rhs=wo[:, c, n2 * nw:(n2 + 1) * nw],
                            start=(c == 0), stop=(c == ncc - 1))
                osb = outp.tile([128, E], F32, tag="osb")
                nc.vector.tensor_copy(out=osb, in_=pd)
                nc.sync.dma_start(out=out[st * 128:(st + 1) * 128, :], in_=osb)

    nc.finalize()
    return nc


def _make_masks(NQ=512):
    """[128, 2*1024] f32: two diag-group masks (d=0,1 | d=2,3)."""
    j = np.arange(128)[:, None]
    i = np.arange(NQ)[None, :]
    blocks = [(j <= i - 128 * d).astype(np.float32) for d in range(4)]
    mA = np.concatenate(blocks[0:2], axis=1)
    mB = np.concatenate(blocks[2:4], axis=1)
    return np.ascontiguousarray(np.concatenate([mA, mB], axis=1))


def _host_prep(x, Wq, Wk, Wv, Wo, HPC=8, DH=64):
    """Build the 8 per-core input maps."""
    B, S, E = x.shape
    nec = E // 128
    HD = HPC * DH
    masks = _make_masks()
    in_maps = []
    for c in range(N_CORES):
        b, g = c // 2, c % 2
        hs = slice(HPC * g, HPC * g + HPC)
        # [h, 2*DH, E] stacked q|k  ->  [128(e%), h, ec, 2*DH]
        wqk = np.concatenate([Wq[hs], Wk[hs]], axis=1)          # [HPC,128,E]
        wqk = wqk.transpose(2, 0, 1).reshape(nec, 128, HPC, 2 * DH)
        wqkt = np.ascontiguousarray(wqk.transpose(1, 2, 0, 3))  # [128,h,ec,m]
        # Wv slice -> [128, ec, HD]
        wvt = Wv[hs].transpose(2, 0, 1).reshape(nec, 128, HD)
        wvt = np.ascontiguousarray(wvt.transpose(1, 0, 2))
        # Wo columns slice, transposed -> [128, ncc, E]
        wot = np.ascontiguousarray(Wo[:, HD * g:HD * (g + 1)].T)  # [HD, E]
        wot = np.ascontiguousarray(
            wot.reshape(HD // 128, 128, E).transpose(1, 0, 2))
        in_maps.append({
            "xb": np.ascontiguousarray(x[b]),
            "wqkt": wqkt, "wvt": wvt, "wot": wot, "maskab": masks,
        })
    return in_maps


_NC_CACHE = {}


def kernel(x, Wq, Wk, Wv, Wo):
    x = np.asarray(x, dtype=np.float32)
    Wq = np.asarray(Wq, dtype=np.float32)
    Wk = np.asarray(Wk, dtype=np.float32)
    Wv = np.asarray(Wv, dtype=np.float32)
    Wo = np.asarray(Wo, dtype=np.float32)
    B, S, E = x.shape
    H, DH, _ = Wq.shape
    HPC = H // 2

    key = (S, E, HPC, DH)
    if key not in _NC_CACHE:
        _NC_CACHE[key] = build_nc(S=S, E=E, HPC=HPC, DH=DH)
    nc = _NC_CACHE[key]

    in_maps = _host_prep(x, Wq, Wk, Wv, Wo, HPC=HPC, DH=DH)
    res = run_bass_kernel_spmd(nc, in_maps, core_ids=list(range(N_CORES)))
    kernel.last_results = res

    out = np.empty((B, S, E), dtype=np.float32)
    for b in range(B):
        out[b] = res.results[2 * b]["out"] + res.results[2 * b + 1]["out"]
    return out


# revision 17
# speedup vs baseline: 1.2032x; 1.2032x over previous
"""Bass/Trainium2 kernel for naive causal multi-head attention.

Problem: B=4, S=2048, E=1024, H=16, DH=64 (fp32 in/out).

Sharding (8 NeuronCores): core c handles batch b = c//2 and head group
g = c%2 (heads 8g..8g+7).  Each core computes its 8 heads' attention for
its batch plus the partial out-projection through its 512 columns of the
concat dim; the host sums the two partial outputs per batch.

Device-side dataflow (all matmuls bf16, PSUM fp32):
  xT   = transpose(cast_bf16(x))                    (cast-DMA + xbar transpose)
  q/kT = Wqk_h @ xT   -> [128(q64|k64), S] per head (PE)
  v'   = x @ Wv^T (+ ones column per head)          (PE)
  sT   = kT_blk^T q  -> [128 keys, 512 q] blocks    (PE, causal-skipped)
  p    = exp(sT/8)   (PSUM->SBUF, diag-masked)      (ACT exp, DVE mask)
  oT   = v'^T p      -> [65, 512]  (row 64 = sums)  (PE, accumulated)
  oT  /= sums        (recip + ones-bcast matmul)    (DVE/PE/ACT)
  out  = concatT^T @ WoT                            (PE)
"""

import os

import numpy as np

import concourse.bacc as bacc
import concourse.mybir as mybir
from concourse.tile import TileContext
from concourse.bass_utils import run_bass_kernel_spmd

F32 = mybir.dt.float32
F32R = mybir.dt.float32r
BF16 = mybir.dt.bfloat16
EXP = mybir.ActivationFunctionType.Exp

N_CORES = 8
NARROW = os.environ.get("K_NARROW", "1") == "1"
OLDNORM = os.environ.get("K_OLDNORM", "0") == "1"


def build_nc(S=2048, E=1024, HPC=8, DH=64):
    """Build the per-core Bass program (identical on all cores)."""
    NQ = 512                      # query-tile width
    nst = S // 128                # s-tiles (key tiles)
    nec = E // 128                # e chunks (contraction tiles)
    nqt = S // NQ                 # query tiles
    HD = HPC * DH                 # local concat width (512)
    ncc = HD // 128               # concat chunks (4)
    nnc = E // 512                # out-proj N chunks (2)
    assert NQ == 512 and S % 512 == 0

    nc = bacc.Bacc("TRN2", target_bir_lowering=False, debug=False,
                   num_devices=N_CORES)

    xb = nc.dram_tensor("xb", [S, E], F32, kind="ExternalInput")
    wqkt = nc.dram_tensor("wqkt", [128, HPC, nec, 2 * DH], F32,
                          kind="ExternalInput")
    wvt = nc.dram_tensor("wvt", [128, nec, HD], F32, kind="ExternalInput")
    wot = nc.dram_tensor("wot", [128, ncc, E], F32, kind="ExternalInput")
    maskab = nc.dram_tensor("maskab", [128, 2 * 1024], F32,
                            kind="ExternalInput")
    out = nc.dram_tensor("out", [S, E], F32, kind="ExternalOutput")

    with TileContext(nc) as tc:
        with (
            tc.tile_pool(name="persist", bufs=1) as persist,
            tc.tile_pool(name="dramp", bufs=1, space="DRAM") as dramp,
            tc.tile_pool(name="qkp", bufs=2) as qkp,
            tc.tile_pool(name="ktp", bufs=2) as ktp,
            tc.tile_pool(name="ptp", bufs=3) as ptp,
            tc.tile_pool(name="outp", bufs=2) as outp,
            tc.tile_pool(name="posbp", bufs=3) as posbp,
            tc.tile_pool(name="miscp", bufs=4) as miscp,
            tc.tile_pool(name="ps_big", bufs=2, space="PSUM") as ps_big,
            tc.tile_pool(name="ps_o", bufs=2, space="PSUM") as ps_o,
            tc.tile_pool(name="ps_sm", bufs=2, space="PSUM") as ps_sm,
        ):
            # ---- persistent SBUF tensors ----
            xT = persist.tile([128, nec, S], BF16)
            wqk = persist.tile([128, HPC, nec, 2 * DH], BF16)
            wv = persist.tile([128, nec, HD], BF16)
            wo = persist.tile([128, ncc, E], BF16)
            vS = persist.tile([128, nst, HPC * (DH + 1)], BF16)
            cT = persist.tile([128, ncc, S], BF16)
            mk = persist.tile([128, 2 * 1024], BF16)
            ones1 = persist.tile([1, DH], BF16)

            # ---- phase A: loads, casts, x transpose (pipelined by s) ----
            nc.vector.memset(ones1, 1.0)
            nc.vector.memset(vS, 1.0)
            xch = []
            for j in range(nqt):
                xbfj = dramp.tile([512, E], BF16, tag=f"xbf{j}", name=f"xbf{j}")
                nc.gpsimd.dma_start(out=xbfj,
                                    in_=xb[j * 512:(j + 1) * 512, :])
                xch.append(xbfj)
            for j in range(nqt):
                for ec in range(nec):
                    nc.sync.dma_start_transpose(
                        out=xT[:, ec, j * 512:(j + 1) * 512],
                        in_=xch[j][:, ec * 128:(ec + 1) * 128])
            nc.gpsimd.dma_start(out=wqk, in_=wqkt[:, :, :, :])
            nc.gpsimd.dma_start(out=wv, in_=wvt[:, :, :])
            nc.gpsimd.dma_start(out=wo, in_=wot[:, :, :])
            nc.gpsimd.dma_start(out=mk, in_=maskab[:, :])

            # ---- phase B: v projection (all heads), with ones column ----
            for st in range(nst):
                pv = ps_sm.tile([128, HD], F32, tag="sm")
                for ec in range(nec):
                    nc.tensor.matmul(
                        pv, lhsT=xT[:, ec, st * 128:(st + 1) * 128],
                        rhs=wv[:, ec], start=(ec == 0), stop=(ec == nec - 1))
                nc.vector.tensor_copy(
                    out=vS[:, st].rearrange("p (h m) -> p h m",
                                            m=DH + 1)[:, :, 0:DH],
                    in_=pv.rearrange("p (h m) -> p h m", m=DH))

            # ---- phase C: per head ----
            for h in range(HPC):
                # C1: q/k projection -> qk [128 (q64|k64), S]
                qk = qkp.tile([128, S], BF16, tag="qk")
                kt_sb = ktp.tile([64, S], BF16, tag="kt")
                for sc in range(nqt):
                    pqk = ps_sm.tile([128, 512], F32, tag="sm")
                    for ec in range(nec):
                        nc.tensor.matmul(
                            pqk, lhsT=wqk[:, h, ec],
                            rhs=xT[:, ec, sc * 512:(sc + 1) * 512],
                            start=(ec == 0), stop=(ec == nec - 1))
                    nc.vector.tensor_copy(
                        out=qk[:, sc * 512:(sc + 1) * 512], in_=pqk)
                    # realign k rows 64:128 -> partitions 0:64, per chunk
                    nc.vector.tensor_copy(
                        out=kt_sb[:, sc * 512:(sc + 1) * 512],
                        in_=qk[64:128, sc * 512:(sc + 1) * 512])

                # C2: attention per query tile
                for qt in range(nqt):
                    po = ps_o.tile([DH + 1, 512], F32, tag="o")
                    ngrp = 2 * qt + 2
                    for g in range(ngrp):
                        ps2 = ps_big.tile([128, 1024], F32, tag="sbig")
                        for kk in range(2):
                            kt = 2 * g + kk
                            d = kt - 4 * qt
                            n0 = (128 * d if d > 0 else 0) if NARROW else 0
                            nc.tensor.matmul(
                                ps2[:, kk * 512 + n0:(kk + 1) * 512],
                                lhsT=kt_sb[:, kt * 128:(kt + 1) * 128],
                                rhs=qk[0:64, qt * 512 + n0:(qt + 1) * 512],
                                start=True, stop=True)
                        pt = ptp.tile([128, 1024], BF16, tag="pt")
                        nc.scalar.activation(out=pt, in_=ps2, func=EXP,
                                             scale=0.125)
                        if g >= 2 * qt:  # diagonal groups need causal mask
                            mi = g - 2 * qt
                            nc.vector.tensor_mul(
                                pt, pt, mk[:, mi * 1024:(mi + 1) * 1024])
                        for kk in range(2):
                            kt = 2 * g + kk
                            d = kt - 4 * qt
                            n0 = (128 * d if d > 0 else 0) if NARROW else 0
                            nc.tensor.matmul(
                                po[:, n0:512],
                                lhsT=vS[:, kt, h * (DH + 1):(h + 1) * (DH + 1)],
                                rhs=pt[:, kk * 512 + n0:(kk + 1) * 512],
                                start=(g == 0 and kk == 0),
                                stop=(g == ngrp - 1 and kk == 1),
                                skip_group_check=True)
                    # evacuate PSUM fast, normalize off the critical path
                    posb = posbp.tile([DH + 1, 512], F32, tag="posb")
                    nc.vector.tensor_copy(out=posb, in_=po)
                    sums0 = miscp.tile([1, 512], F32, tag="sums0")
                    nc.vector.tensor_copy(out=sums0, in_=po[DH:DH + 1, :])
                    if OLDNORM:
                        recb = miscp.tile([1, 512], BF16, tag="recb")
                        with nc.allow_low_precision("softmax recip bf16"):
                            nc.vector.reciprocal(recb, sums0)
                    else:
                        rec = miscp.tile([1, 512], F32, tag="rec")
                        nc.vector.reciprocal_approx_fast(
                            out=rec, in_=sums0)
                        recb = miscp.tile([1, 512], BF16, tag="recb")
                        nc.vector.tensor_copy(out=recb, in_=rec)
                    pb = ps_sm.tile([64, 512], F32, tag="sm")
                    nc.tensor.matmul(pb, lhsT=ones1, rhs=recb,
                                     start=True, stop=True)
                    bc = miscp.tile([64, 512], BF16, tag="bc")
                    nc.scalar.copy(bc, pb)
                    nc.vector.tensor_mul(
                        cT[64 * (h % 2):64 * (h % 2) + 64, h // 2,
                           qt * 512 + 0:(qt + 1) * 512],
                        posb[0:DH, :], bc)

            # ---- phase D: partial out-projection ----
            nw = min(512, E)
            for st in range(nst):
                pd = ps_big.tile([128, min(E, 1024)], F32, tag="sbig")
                for n2 in range(E // nw):
                    for c in range(ncc):
                        nc.tensor.matmul(
                            pd[:, n2 * nw:(n2 + 1) * nw],
                            lhsT=cT[:, c, st * 128:(st + 1) * 128],
                            rhs=wo[:, c, n2 * nw:(n2 + 1) * nw],
                            start=(c == 0), stop=(c == ncc - 1))
                osb = outp.tile([128, E], F32, tag="osb")
                nc.scalar.copy(osb, pd)
                nc.sync.dma_start(out=out[st * 128:(st + 1) * 128, :], in_=osb)

    nc.finalize()
    return nc


def _make_masks(NQ=512):
    """[128, 2*1024] f32: two diag-group masks (d=0,1 | d=2,3)."""
    j = np.arange(128)[:, None]
    i = np.arange(NQ)[None, :]
    blocks = [(j <= i - 128 * d).astype(np.float32) for d in range(4)]
    mA = np.concatenate(blocks[0:2], axis=1)
    mB = np.concatenate(blocks[2:4], axis=1)
    return np.ascontiguousarray(np.concatenate([mA, mB], axis=1))


def _host_prep(x, Wq, Wk, Wv, Wo, HPC=8, DH=64):
    """Build the 8 per-core input maps."""
    B, S, E = x.shape
    nec = E // 128
    HD = HPC * DH
    masks = _make_masks()
    in_maps = []
    for c in range(N_CORES):
        b, g = c // 2, c % 2
        hs = slice(HPC * g, HPC * g + HPC)
        # [h, 2*DH, E] stacked q|k  ->  [128(e%), h, ec, 2*DH]
        wqk = np.concatenate([Wq[hs], Wk[hs]], axis=1)          # [HPC,128,E]
        wqk = wqk.transpose(2, 0, 1).reshape(nec, 128, HPC, 2 * DH)
        wqkt = np.ascontiguousarray(wqk.transpose(1, 2, 0, 3))  # [128,h,ec,m]
        # Wv slice -> [128, ec, HD]
        wvt = Wv[hs].transpose(2, 0, 1).reshape(nec, 128, HD)
        wvt = np.ascontiguousarray(wvt.transpose(1, 0, 2))
        # Wo columns slice, transposed -> [128, ncc, E]
        wot = np.ascontiguousarray(Wo[:, HD * g:HD * (g + 1)].T)  # [HD, E]
        wot = np.ascontiguousarray(
            wot.reshape(HD // 128, 128, E).transpose(1, 0, 2))
        in_maps.append({
            "xb": np.ascontiguousarray(x[b]),
            "wqkt": wqkt, "wvt": wvt, "wot": wot, "maskab": masks,
        })
    return in_maps


_NC_CACHE = {}


def kernel(x, Wq, Wk, Wv, Wo):
    x = np.asarray(x, dtype=np.float32)
    Wq = np.asarray(Wq, dtype=np.float32)
    Wk = np.asarray(Wk, dtype=np.float32)
    Wv = np.asarray(Wv, dtype=np.float32)
    Wo = np.asarray(Wo, dtype=np.float32)
    B, S, E = x.shape
    H, DH, _ = Wq.shape
    HPC = H // 2

    key = (S, E, HPC, DH)
    if key not in _NC_CACHE:
        _NC_CACHE[key] = build_nc(S=S, E=E, HPC=HPC, DH=DH)
    nc = _NC_CACHE[key]

    in_maps = _host_prep(x, Wq, Wk, Wv, Wo, HPC=HPC, DH=DH)
    res = run_bass_kernel_spmd(nc, in_maps, core_ids=list(range(N_CORES)))
    kernel.last_results = res

    out = np.empty((B, S, E), dtype=np.float32)
    for b in range(B):
        out[b] = res.results[2 * b]["out"] + res.results[2 * b + 1]["out"]
    return out


# revision 18
# speedup vs baseline: 1.7213x; 1.4306x over previous
"""Bass/Trainium2 kernel for naive causal multi-head attention.

Problem: B=4, S=2048, E=1024, H=16, DH=64 (fp32 in/out).

Sharding (8 NeuronCores): core c handles batch b = c//2 and head group
g = c%2 (heads 8g..8g+7).  Each core computes its 8 heads' attention for
its batch plus the partial out-projection through its 512 columns of the
concat dim; the host sums the two partial outputs per batch.

Device-side dataflow (all matmuls bf16, PSUM fp32):
  xT   = transpose(cast_bf16(x))                    (cast-DMA + xbar transpose)
  q/kT = Wqk_h @ xT   -> [128(q64|k64), S] per head (PE)
  v'   = x @ Wv^T (+ ones column per head)          (PE)
  sT   = kT_blk^T q  -> [128 keys, 512 q] blocks    (PE, causal-skipped)
  p    = exp(sT/8)   (PSUM->SBUF, diag-masked)      (ACT exp, DVE mask)
  oT   = v'^T p      -> [65, 512]  (row 64 = sums)  (PE, accumulated)
  oT  /= sums        (recip + ones-bcast matmul)    (DVE/PE/ACT)
  out  = concatT^T @ WoT                            (PE)
"""

import os

import numpy as np

import concourse.bacc as bacc
import concourse.bass as bass
import concourse.mybir as mybir
from concourse.tile import TileContext
from concourse.bass_utils import run_bass_kernel_spmd

F32 = mybir.dt.float32
F32R = mybir.dt.float32r
BF16 = mybir.dt.bfloat16
EXP = mybir.ActivationFunctionType.Exp

N_CORES = 8
NARROW = os.environ.get("K_NARROW", "1") == "1"
OLDNORM = os.environ.get("K_OLDNORM", "0") == "1"


def build_nc(S=2048, E=1024, HPC=8, DH=64):
    """Build the per-core Bass program (identical on all cores)."""
    NQ = 512                      # query-tile width
    nst = S // 128                # s-tiles (key tiles)
    nec = E // 128                # e chunks (contraction tiles)
    nqt = S // NQ                 # query tiles
    HD = HPC * DH                 # local concat width (512)
    ncc = HD // 128               # concat chunks (4)
    nnc = E // 512                # out-proj N chunks (2)
    assert NQ == 512 and S % 512 == 0

    nc = bacc.Bacc("TRN2", target_bir_lowering=False, debug=False,
                   num_devices=N_CORES)

    xb = nc.dram_tensor("xb", [S, E], F32, kind="ExternalInput")
    wqkt = nc.dram_tensor("wqkt", [128, HPC, nec, 2 * DH], F32,
                          kind="ExternalInput")
    wvt = nc.dram_tensor("wvt", [128, nec, HD], F32, kind="ExternalInput")
    wot = nc.dram_tensor("wot", [128, ncc, E], F32, kind="ExternalInput")
    maskab = nc.dram_tensor("maskab", [128, 2 * 1024], F32,
                            kind="ExternalInput")
    out = nc.dram_tensor("out", [S, E], F32, kind="ExternalOutput")

    with TileContext(nc) as tc:
        with (
            tc.tile_pool(name="persist", bufs=1) as persist,
            tc.tile_pool(name="dramp", bufs=1, space="DRAM") as dramp,
            tc.tile_pool(name="qkp", bufs=2) as qkp,
            tc.tile_pool(name="ktp", bufs=2) as ktp,
            tc.tile_pool(name="ptp", bufs=3) as ptp,
            tc.tile_pool(name="outp", bufs=2) as outp,
            tc.tile_pool(name="posbp", bufs=3) as posbp,
            tc.tile_pool(name="miscp", bufs=4) as miscp,
            tc.tile_pool(name="ps_big", bufs=2, space="PSUM") as ps_big,
            tc.tile_pool(name="ps_o", bufs=2, space="PSUM") as ps_o,
            tc.tile_pool(name="ps_sm", bufs=2, space="PSUM") as ps_sm,
        ):
            # ---- persistent SBUF tensors ----
            xT = persist.tile([128, nec, S], BF16)
            wqk = persist.tile([128, HPC, nec, 2 * DH], BF16)
            wv = persist.tile([128, nec, HD], BF16)
            wo = persist.tile([128, ncc, E], BF16)
            vS = persist.tile([128, nst, HPC * (DH + 1)], BF16)
            cT = persist.tile([128, ncc, S], BF16)
            mk = persist.tile([128, 2 * 1024], BF16)
            ones1 = persist.tile([1, DH], BF16)

            # ---- phase A: weights first, then x streamed per chunk ----
            nc.vector.memset(ones1, 1.0)
            nc.vector.memset(vS, 1.0)
            nc.gpsimd.dma_start(out=wv, in_=wvt[:, :, :])
            nc.gpsimd.dma_start(out=wqk, in_=wqkt[:, :, :, :])
            nc.gpsimd.dma_start(out=mk, in_=maskab[:, :])
            nc.gpsimd.dma_start(out=wo, in_=wot[:, :, :])
            # per 512-token chunk: cast -> transpose -> v projection
            for j in range(nqt):
                xbfj = dramp.tile([512, E], BF16, tag=f"xbf{j}", name=f"xbf{j}")
                nc.gpsimd.dma_start(out=xbfj,
                                    in_=xb[j * 512:(j + 1) * 512, :])
                for ec in range(nec):
                    nc.sync.dma_start_transpose(
                        out=xT[:, ec, j * 512:(j + 1) * 512],
                        in_=xbfj[:, ec * 128:(ec + 1) * 128])
                for st in range(4 * j, 4 * j + 4):
                    pv = ps_sm.tile([128, HD], F32, tag="sm")
                    for ec in range(nec):
                        nc.tensor.matmul(
                            pv, lhsT=xT[:, ec, st * 128:(st + 1) * 128],
                            rhs=wv[:, ec], start=(ec == 0),
                            stop=(ec == nec - 1))
                    nc.vector.tensor_copy(
                        out=vS[:, st].rearrange("p (h m) -> p h m",
                                                m=DH + 1)[:, :, 0:DH],
                        in_=pv.rearrange("p (h m) -> p h m", m=DH))

            # ---- phase C: per head ----
            for h in range(HPC):
                # C1: q/k projection -> qk [128 (q64|k64), S]
                qk = qkp.tile([128, S], BF16, tag="qk")
                kt_sb = ktp.tile([64, S], BF16, tag="kt")
                for sc in range(nqt):
                    pqk = ps_sm.tile([128, 512], F32, tag="sm")
                    for ec in range(nec):
                        nc.tensor.matmul(
                            pqk, lhsT=wqk[:, h, ec],
                            rhs=xT[:, ec, sc * 512:(sc + 1) * 512],
                            start=(ec == 0), stop=(ec == nec - 1))
                    nc.vector.tensor_copy(
                        out=qk[:, sc * 512:(sc + 1) * 512], in_=pqk)
                    # realign k rows 64:128 -> partitions 0:64, per chunk
                    nc.vector.tensor_copy(
                        out=kt_sb[:, sc * 512:(sc + 1) * 512],
                        in_=qk[64:128, sc * 512:(sc + 1) * 512])

                # C2: attention per query tile
                for qt in range(nqt):
                    po = ps_o.tile([DH + 1, 512], F32, tag="o")
                    ngrp = 2 * qt + 2
                    for g in range(ngrp):
                        ps2 = ps_big.tile([128, 1024], F32, tag="sbig")
                        for kk in range(2):
                            kt = 2 * g + kk
                            d = kt - 4 * qt
                            n0 = (128 * d if d > 0 else 0) if NARROW else 0
                            nc.tensor.matmul(
                                ps2[:, kk * 512 + n0:(kk + 1) * 512],
                                lhsT=kt_sb[:, kt * 128:(kt + 1) * 128],
                                rhs=qk[0:64, qt * 512 + n0:(qt + 1) * 512],
                                start=True, stop=True)
                        pt = ptp.tile([128, 1024], BF16, tag="pt")
                        nc.scalar.activation(out=pt, in_=ps2, func=EXP,
                                             scale=0.125)
                        if g >= 2 * qt:  # diagonal groups need causal mask
                            mi = g - 2 * qt
                            nc.vector.tensor_mul(
                                pt, pt, mk[:, mi * 1024:(mi + 1) * 1024])
                        for kk in range(2):
                            kt = 2 * g + kk
                            d = kt - 4 * qt
                            n0 = (128 * d if d > 0 else 0) if NARROW else 0
                            nc.tensor.matmul(
                                po[:, n0:512],
                                lhsT=vS[:, kt, h * (DH + 1):(h + 1) * (DH + 1)],
                                rhs=pt[:, kk * 512 + n0:(kk + 1) * 512],
                                start=(g == 0 and kk == 0),
                                stop=(g == ngrp - 1 and kk == 1),
                                skip_group_check=True)
                    # evacuate PSUM fast, normalize off the critical path
                    posb = posbp.tile([DH + 1, 512], F32, tag="posb")
                    nc.vector.tensor_copy(out=posb, in_=po)
                    sums0 = miscp.tile([1, 512], F32, tag="sums0")
                    nc.vector.tensor_copy(out=sums0, in_=po[DH:DH + 1, :])
                    rec = miscp.tile([1, 512], F32, tag="rec")
                    nc.vector.reciprocal_approx_fast(out=rec, in_=sums0)
                    recd = dramp.tile([1, 512], F32, tag="recd", bufs=4)
                    nc.sync.dma_start(out=recd, in_=rec)
                    bc = miscp.tile([64, 512], F32, tag="bc")
                    nc.sync.dma_start(
                        out=bc,
                        in_=bass.AP(tensor=recd.tensor, offset=recd.offset,
                                    ap=[[0, 64]] + list(recd.ap[1:])))
                    nc.vector.tensor_mul(
                        cT[64 * (h % 2):64 * (h % 2) + 64, h // 2,
                           qt * 512 + 0:(qt + 1) * 512],
                        posb[0:DH, :], bc)

            # ---- phase D: partial out-projection ----
            nw = min(512, E)
            for st in range(nst):
                pd = ps_big.tile([128, min(E, 1024)], F32, tag="sbig")
                for n2 in range(E // nw):
                    for c in range(ncc):
                        nc.tensor.matmul(
                            pd[:, n2 * nw:(n2 + 1) * nw],
                            lhsT=cT[:, c, st * 128:(st + 1) * 128],
                            rhs=wo[:, c, n2 * nw:(n2 + 1) * nw],
                            start=(c == 0), stop=(c == ncc - 1))
                osb = outp.tile([128, E], F32, tag="osb")
                nc.scalar.copy(osb, pd)
                nc.sync.dma_start(out=out[st * 128:(st + 1) * 128, :], in_=osb)

    nc.finalize()
    return nc


def _make_masks(NQ=512):
    """[128, 2*1024] f32: two diag-group masks (d=0,1 | d=2,3)."""
    j = np.arange(128)[:, None]
    i = np.arange(NQ)[None, :]
    blocks = [(j <= i - 128 * d).astype(np.float32) for d in range(4)]
    mA = np.concatenate(blocks[0:2], axis=1)
    mB = np.concatenate(blocks[2:4], axis=1)
    return np.ascontiguousarray(np.concatenate([mA, mB], axis=1))


def _host_prep(x, Wq, Wk, Wv, Wo, HPC=8, DH=64):
    """Build the 8 per-core input maps."""
    B, S, E = x.shape
    nec = E // 128
    HD = HPC * DH
    masks = _make_masks()
    in_maps = []
    for c in range(N_CORES):
        b, g = c // 2, c % 2
        hs = slice(HPC * g, HPC * g + HPC)
        # [h, 2*DH, E] stacked q|k  ->  [128(e%), h, ec, 2*DH]
        wqk = np.concatenate([Wq[hs], Wk[hs]], axis=1)          # [HPC,128,E]
        wqk = wqk.transpose(2, 0, 1).reshape(nec, 128, HPC, 2 * DH)
        wqkt = np.ascontiguousarray(wqk.transpose(1, 2, 0, 3))  # [128,h,ec,m]
        # Wv slice -> [128, ec, HD]
        wvt = Wv[hs].transpose(2, 0, 1).reshape(nec, 128, HD)
        wvt = np.ascontiguousarray(wvt.transpose(1, 0, 2))
        # Wo columns slice, transposed -> [128, ncc, E]
        wot = np.ascontiguousarray(Wo[:, HD * g:HD * (g + 1)].T)  # [HD, E]
        wot = np.ascontiguousarray(
            wot.reshape(HD // 128, 128, E).transpose(1, 0, 2))
        in_maps.append({
            "xb": np.ascontiguousarray(x[b]),
            "wqkt": wqkt, "wvt": wvt, "wot": wot, "maskab": masks,
        })
    return in_maps


_NC_CACHE = {}


def kernel(x, Wq, Wk, Wv, Wo):
    x = np.asarray(x, dtype=np.float32)
    Wq = np.asarray(Wq, dtype=np.float32)
    Wk = np.asarray(Wk, dtype=np.float32)
    Wv = np.asarray(Wv, dtype=np.float32)
    Wo = np.asarray(Wo, dtype=np.float32)
    B, S, E = x.shape
    H, DH, _ = Wq.shape
    HPC = H // 2

    key = (S, E, HPC, DH)
    if key not in _NC_CACHE:
        _NC_CACHE[key] = build_nc(S=S, E=E, HPC=HPC, DH=DH)
    nc = _NC_CACHE[key]

    in_maps = _host_prep(x, Wq, Wk, Wv, Wo, HPC=HPC, DH=DH)
    res = run_bass_kernel_spmd(nc, in_maps, core_ids=list(range(N_CORES)))
    kernel.last_results = res

    out = np.empty((B, S, E), dtype=np.float32)
    for b in range(B):
        out[b] = res.results[2 * b]["out"] + res.results[2 * b + 1]["out"]
    return out
